# revision 28
# baseline (speedup 1.0000x reference)
"""Trainium2 8-core kernel for the Contrast module (fp8 DoubleRow + split AG).

    za_p = ELU(za @ W1 + b1) @ W2 + b2          (same for zb)
    za_ca = softmax((za_p Wq + bq)(zb_p Wk + bk)^T / sqrt(256)) @ (zb_p Wv + bv)
    zb_ca = softmax((zb_p Wq + bq)(za_p Wk + bk)^T / sqrt(256)) @ (za_p Wv + bv)
    out = concat(za_ca, zb_ca, axis=1)

Key structural choices:
 - Wk is folded into the query side on the host: with Wqk = Wq Wk^T / s and
   bqk = bq Wk^T / s, softmax(Q K^T/s) == softmax((P_q Wqk + bqk) P_k^T)
   (the dropped (P_q Wq + bq)·bk^T term is constant along k, so it cancels
   in softmax). This removes the entire K projection: the score matmul
   contracts Q'' (fp8) against the raw projections P_k (fp8).
 - fp8 scaling: Wqk carries 1/4 and the P_k fp8 copy carries 1/4 (product
   1/16 = 1/sqrt(D/2)); splitting the scale keeps both operands out of the
   fp8e4 subnormal range.
 - TWO AllGathers, one per attention direction, each fully overlapped:
   AG(dir b) ships P_a/V_a during the zb projection; AG(dir a) ships
   P_b/V_b during direction-b attention. Neither exposes collective time
   on the PE critical path (the single-AG variant stalled PE ~80us).
 - Attention matmuls (scores and attn@V) run in fp8 DoubleRow perf mode;
   PSUM accumulation stays f32.
 - Projections (W1/ELU/W2, V, Q'') stay f32r for accuracy.

Sharding: rows data-parallel across 8 cores; weights replicated.
Layout notes:
 - Activations flow feature-major ("transposed"): zaT [h, n] so every
   matmul contracts over the partition axis without on-chip transposes.
 - softmax denominator: V panels carry a ones column; attn@V is split into
   N=256 and N=258 matmuls so the rowsum accumulates in PSUM col 512.
 - No max-subtraction in softmax: scores are ~N(0, 0.85^2); exp(s-2) keeps
   fp8 exp outputs in range, and the e^-2 cancels against the denominator.
 - ELU+1 = max(x+1, min(exp(x), 1)) via one Act + two DVE ops
   (scalar_tensor_tensor fuses the min+max); the -1 is folded into b2.
 - bv is added at finalize (softmax weights sum to 1).
"""

import numpy as np

import concourse.mybir as mybir
import concourse.tile as tile
from concourse import bacc
from concourse.bass_utils import run_bass_kernel_spmd

dt = mybir.dt
AF = mybir.ActivationFunctionType
ALU = mybir.AluOpType
PM = mybir.MatmulPerfMode

R = 8            # cores
N = 8192         # total rows
H = 1024         # hidden
D = 512          # attention dim
NL = N // R      # rows per core
HC = H // 128    # 8 h-chunks
DC = D // 128    # 4 d-chunks
NB = NL // 512   # 2 n-blocks per core slice
KC = NL // 128   # 8 key-chunks per shard
SCALE = 16.0     # sqrt(512/2)
QS = 4.0         # per-operand fp8 scale split: (Q''/4)·(P/4) = Q''P/16
KVF = D * NL     # elements per P (or V) shard section
SHARD = 2 * KVF  # P_x | V_x, fp8 bytes per core per direction

F32R = dt.float32r
BF16 = dt.bfloat16
F8 = dt.float8e4


def _r(ap):
    return ap.bitcast(F32R)


def build():
    nc = bacc.Bacc("TRN2", target_bir_lowering=False, debug=False, num_devices=R)

    def inp(name, shape, dtype=dt.float32):
        return nc.dram_tensor(name, shape, dtype, kind="ExternalInput")

    zT8 = {
        "a": inp("zaT8", [128, HC // 2, NL], F8),
        "b": inp("zbT8", [128, HC // 2, NL], F8),
    }
    zTb = {
        "a": inp("zaTb", [128, HC // 2, NL], BF16),
        "b": inp("zbTb", [128, HC // 2, NL], BF16),
    }
    w18 = inp("W1t8", [128, HC // 2, H], F8)
    w1b = inp("W1tb", [128, HC // 2, H], BF16)
    w2 = inp("W2t", [128, HC, D])
    wqk = inp("Wqkt", [128, DC, D], BF16)
    wv = inp("Wvt", [128, DC, D], BF16)
    b1d = inp("b1t", [128, HC])
    b2d = inp("b2t", [128, DC])
    bqkd = inp("bqkt", [128, DC])
    bvd = inp("bvt", [128, D])
    vpadd = inp("vpad", [128, 2 * KC], F8)
    out_d = nc.dram_tensor("out", [NL, 2 * D], dt.float32, kind="ExternalOutput")

    with tile.TileContext(nc) as tc:
        psum = tc.alloc_tile_pool(name="psum", bufs=1, space="PSUM")
        dram = tc.alloc_tile_pool(name="dram", bufs=1, space="DRAM")
        const = tc.alloc_tile_pool(name="const", bufs=1)
        qtp = tc.alloc_tile_pool(name="qtp", bufs=1)
        wkvp = tc.alloc_tile_pool(name="wkvp", bufs=1)
        projp = tc.alloc_tile_pool(name="projp", bufs=1)

        # ---- DMAs ordered by first PE use: (w1t[hc], z[hc]) pairs so the
        # hc-accumulation of the first W1 block starts as soon as each pair
        # lands; biases next (first ELU needs them only ~2us in).
        w1t8 = projp.tile([128, HC // 2, H], F8, name="w1t8")
        w1tb = projp.tile([128, HC // 2, H], BF16, name="w1tb")
        zt = {}
        b1 = const.tile([128, HC], dt.float32, name="b1")
        b2 = const.tile([128, DC], dt.float32, name="b2")
        bqk = const.tile([128, DC], dt.float32, name="bqk")
        bv = const.tile([128, D], dt.float32, name="bv")
        z0_8 = projp.tile([128, HC // 2, 512], F8, tag="z8", bufs=2, name="z8_a0")
        z0_b = projp.tile([128, HC // 2, 512], BF16, tag="zb", bufs=2, name="zb_a0")
        zt[("a", 0)] = (z0_8, z0_b)
        for hc in range(HC // 2):
            nc.sync.dma_start(w1t8[:, hc, :], w18.ap()[:, hc, :])
            nc.scalar.dma_start(w1tb[:, hc, :], w1b.ap()[:, hc, :])
            nc.gpsimd.dma_start(z0_8[:, hc, :], zT8["a"].ap()[:, hc, 0:512])
            nc.gpsimd.dma_start(z0_b[:, hc, :], zTb["a"].ap()[:, hc, 0:512])
        nc.sync.dma_start(b1[:], b1d.ap())
        z1_8 = projp.tile([128, HC // 2, 512], F8, tag="z8", bufs=2, name="z8_a1")
        z1_b = projp.tile([128, HC // 2, 512], BF16, tag="zb", bufs=2, name="zb_a1")
        zt[("a", 1)] = (z1_8, z1_b)
        for hc in range(HC // 2):
            nc.gpsimd.dma_start(z1_8[:, hc, :], zT8["a"].ap()[:, hc, 512:1024])
            nc.gpsimd.dma_start(z1_b[:, hc, :], zTb["a"].ap()[:, hc, 512:1024])
        w2t = projp.tile([128, HC, D], F32R, name="w2t")
        nc.sync.dma_start(b2[:], b2d.ap())
        nc.sync.dma_start(w2t[:], _r(w2.ap()))
        wqkt = wkvp.tile([128, DC, D], BF16, name="wqkt")
        wvt = wkvp.tile([128, DC, D], BF16, name="wvt")
        nc.sync.dma_start(wvt[:], wv.ap())
        nc.sync.dma_start(wqkt[:], wqk.ap())
        nc.sync.dma_start(bqk[:], bqkd.ap())
        nc.sync.dma_start(bv[:], bvd.ap())
        # exp(x - 2): keeps fp8 exp outputs in range for scores up to ~7.4;
        # the e^-2 cancels between numerator and the ones-column denominator.
        negc = const.tile([128, 1], dt.float32, name="negc")
        nc.vector.memset(negc[:], -2.0)
        # z(b) chunk DMAs issued up front on the gpsimd ring; the z-tag
        # rotation (bufs=2) gates each write on the a-block consumers, so
        # they stream in as soon as the W1(a) matmuls retire.
        for nb in range(NB):
            zb8 = projp.tile([128, HC // 2, 512], F8, tag="z8", bufs=2, name=f"z8_b{nb}")
            zbb = projp.tile([128, HC // 2, 512], BF16, tag="zb", bufs=2, name=f"zb_b{nb}")
            zt[("b", nb)] = (zb8, zbb)
            for hc in range(HC // 2):
                nc.gpsimd.dma_start(
                    zb8[:, hc, :], zT8["b"].ap()[:, hc, nb * 512 : (nb + 1) * 512]
                )
                nc.gpsimd.dma_start(
                    zbb[:, hc, :], zTb["b"].ap()[:, hc, nb * 512 : (nb + 1) * 512]
                )

        pT = {
            "a": wkvp.tile([128, DC, NL], BF16, name="pta"),
            "b": wkvp.tile([128, DC, NL], BF16, name="ptb"),
        }

        # per-direction fused AG buffers: [P_x | V_x] fp8
        agin = {x: dram.tile([SHARD], F8, name=f"agin_{x}") for x in ("b", "a")}
        agout = {
            x: dram.tile([R * SHARD], F8, name=f"agout_{x}", addr_space="Shared")
            for x in ("b", "a")
        }

        # ================= projection + P/V shards =================
        def mmtile(g=None):
            """single-bank accumulation tile; 5 rotating buffers so the PE
            never waits on a lagging Act/DVE consumer."""
            return psum.tile([128, 512], dt.float32, tag="mm", bufs=4, name="ps1")

        for src, other in (("a", "b"), ("b", "a")):
            for nb in range(NB):
                ns = slice(nb * 512, (nb + 1) * 512)
                z8, zb16 = zt[(src, nb)]
                hT = projp.tile([128, HC, 512], F32R, tag="h", bufs=2, name=f"h_{src}{nb}")
                # ELU(x)+1 = max(x+1, min(exp(x), 1)), x = ps + b1.
                # The combine (stt) is software-pipelined one block behind so
                # the DVE's PSUM reader (xp1) is never queued behind a combine
                # and the PSUM slot frees right after Act's Exp + DVE's add.
                stt_q = []
                for d1c in range(HC):
                    ps = mmtile()
                    for c in range(HC // 4):
                        nc.tensor.matmul(
                            ps,
                            w1t8[:, 2 * c : 2 * c + 2, d1c * 128 : (d1c + 1) * 128],
                            z8[:, 2 * c : 2 * c + 2, :],
                            start=(c == 0),
                            stop=False,
                            perf_mode=PM.DoubleRow,
                        )
                    for hc in range(HC // 2):
                        nc.tensor.matmul(
                            ps,
                            w1tb[:, hc, d1c * 128 : (d1c + 1) * 128],
                            zb16[:, hc, :],
                            start=False,
                            stop=(hc == HC // 2 - 1),
                        )
                    e = projp.tile([128, 512], dt.float32, tag="e", bufs=3, name="e")
                    xp1 = projp.tile([128, 512], dt.float32, tag="xp1", bufs=3, name="xp1")
                    # W1 shipped x16 in fp8; fold the 1/16 into the Act scale
                    nc.scalar.activation(
                        e[:], ps, AF.Exp, bias=b1[:, d1c : d1c + 1], scale=1.0 / 16
                    )
                    nc.scalar.activation(
                        xp1[:], ps, AF.Relu, bias=b1[:, d1c : d1c + 1], scale=1.0 / 16
                    )
                    stt_q.append((d1c, e, xp1))
                    if len(stt_q) > 1:
                        dd, ee, xx = stt_q.pop(0)
                        nc.vector.scalar_tensor_tensor(
                            hT[:, dd, :], ee[:], 1.0, xx[:], ALU.min, ALU.add
                        )
                dd, ee, xx = stt_q.pop(0)
                nc.vector.scalar_tensor_tensor(
                    hT[:, dd, :], ee[:], 1.0, xx[:], ALU.min, ALU.add
                )
                for d2c in range(DC):
                    ps = mmtile()
                    for d1c in range(HC):
                        nc.tensor.matmul(
                            ps,
                            w2t[:, d1c, d2c * 128 : (d2c + 1) * 128],
                            hT[:, d1c, :],
                            start=(d1c == 0),
                            stop=(d1c == HC - 1),
                        )
                    nc.scalar.activation(
                        pT[src][:, d2c, ns], ps, AF.Identity, bias=b2[:, d2c : d2c + 1]
                    )

            # stage this src's P (fp8, /4) + V into the *other* direction's
            # AG shard, then fire that direction's AllGather.
            pv = agin[other][0:KVF].rearrange("(d n) -> d n", n=NL)
            vv = agin[other][KVF : 2 * KVF].rearrange("(n d) -> n d", d=D)
            for dc in range(DC):
                for nb in range(NB):
                    ns = slice(nb * 512, (nb + 1) * 512)
                    s = projp.tile([128, 512], F8, tag="stg", bufs=16, name="stg_p")
                    nc.vector.tensor_scalar(
                        s[:], pT[src][:, dc, ns], 1.0 / QS, None, ALU.mult
                    )
                    nc.sync.dma_start(pv[dc * 128 : (dc + 1) * 128, ns], s[:])
            for nt in range(KC):
                ps = mmtile()
                for d2c in range(DC):
                    nc.tensor.matmul(
                        ps,
                        pT[src][:, d2c, nt * 128 : (nt + 1) * 128],
                        wvt[:, d2c, :],
                        start=(d2c == 0),
                        stop=(d2c == DC - 1),
                    )
                s = projp.tile([128, 512], F8, tag="stg", bufs=16, name="stg_v")
                nc.scalar.activation(s[:], ps, AF.Copy)
                nc.sync.dma_start(vv[nt * 128 : (nt + 1) * 128, :], s[:])
            nc.gpsimd.collective_compute(
                "AllGather",
                ALU.bypass,
                ins=[agin[other].opt()],
                outs=[agout[other].opt()],
                replica_groups=[list(range(R))],
            )

        # Pre-load direction-b shards r=0,1 into FRESH SBUF (attnp pool
        # created while projp is still live, so these tiles do not overlap
        # released projection space — otherwise their DMAs inherit an
        # anti-dependency on the last Q'' matmul and attention starts late).
        attnp = tc.alloc_tile_pool(name="attnp", bufs=1)
        pre = {}

        def load_shard(pool, x, r, eng=None):
            eng = eng or nc.sync
            base = r * SHARD
            ktile = pool.tile([128, DC, NL], F8, tag="kt", bufs=2, name=f"kt{r}")
            eng.dma_start(
                ktile[:],
                agout[x][base : base + KVF].rearrange(
                    "(dc p n) -> p dc n", p=128, n=NL
                ),
            )
            vtile = pool.tile([128, KC, D + 2], F8, tag="vt", bufs=3, name=f"vt{r}")
            eng.dma_start(
                vtile[:, :, 0:D],
                agout[x][base + KVF : base + 2 * KVF].rearrange(
                    "(kc p d) -> p kc d", p=128, d=D
                ),
            )
            eng.dma_start(
                vtile[:, :, D : D + 2],
                vpadd.ap().rearrange("p (kc c) -> p kc c", c=2),
            )
            return ktile, vtile

        for r in (0, 1):
            pre[("b", r)] = load_shard(attnp, "b", r, eng=nc.gpsimd)

        # ================= queries Q'' =================
        # Q''(b) is needed right away; Q''(a) is deferred until just before
        # direction-a attention so direction-b attention starts ~8us sooner.
        qT = {}

        def q_proj(x):
            qT[x] = qtp.tile([128, DC, NL], F8, name=f"qt_{x}")
            for dc in range(DC):
                for nb in range(NB):
                    ps = mmtile()
                    for d2c in range(DC):
                        nc.tensor.matmul(
                            ps,
                            wqkt[:, d2c, dc * 128 : (dc + 1) * 128],
                            pT[x][:, d2c, nb * 512 : (nb + 1) * 512],
                            start=(d2c == 0),
                            stop=(d2c == DC - 1),
                        )
                    nc.scalar.activation(
                        qT[x][:, dc, nb * 512 : (nb + 1) * 512],
                        ps,
                        AF.Identity,
                        bias=bqk[:, dc : dc + 1],
                    )

        q_proj("b")

        # ================= attention (fp8 DoubleRow) =================
        # Software-pipelined: block i+1's score matmuls are issued before
        # block i's attn@V so the Exp activations never wait on PE.
        accs = {}
        pending = []  # deferred attn@V closures, one per (x, r, qb) block

        def do_scores(x, r, qb, ktile):
            qs = slice(qb * 512, (qb + 1) * 512)
            exps = []
            for kp in range(KC // 2):
                ex = attnp.tile([128, 2, 512], F8, tag="exp", bufs=16, name="ex")
                for j in range(2):
                    kt_i = 2 * kp + j
                    ps = mmtile()
                    for c in range(DC // 2):
                        nc.tensor.matmul(
                            ps,
                            ktile[:, 2 * c : 2 * c + 2,
                                  kt_i * 128 : (kt_i + 1) * 128],
                            qT[x][:, 2 * c : 2 * c + 2, qs],
                            start=(c == 0),
                            stop=(c == DC // 2 - 1),
                            perf_mode=PM.DoubleRow,
                        )
                    nc.scalar.activation(ex[:, j, :], ps, AF.Exp, bias=negc[:])
                exps.append(ex)
            return exps

        def do_attnv(x, col, r, qb, exps, vtile):
            for qt_i in range(4):
                qsl = slice(qt_i * 128, (qt_i + 1) * 128)
                p1 = psum.tile([128, 256], dt.float32, tag="po1", bufs=2, name="po1")
                p2 = psum.tile([128, 258], dt.float32, tag="po2", bufs=2, name="po2")
                for kp in range(KC // 2):
                    nc.tensor.matmul(
                        p1[:],
                        exps[kp][:, :, qsl],
                        vtile[:, 2 * kp : 2 * kp + 2, 0:256],
                        start=(kp == 0),
                        stop=(kp == KC // 2 - 1),
                        perf_mode=PM.DoubleRow,
                    )
                    nc.tensor.matmul(
                        p2[:],
                        exps[kp][:, :, qsl],
                        vtile[:, 2 * kp : 2 * kp + 2, 256 : D + 2],
                        start=(kp == 0),
                        stop=(kp == KC // 2 - 1),
                        perf_mode=PM.DoubleRow,
                    )
                if r == 0:
                    acc = attnp.tile(
                        [128, D + 2], dt.float32, tag="acc", bufs=8,
                        name=f"acc{qb}{qt_i}",
                    )
                    accs[(x, qb, qt_i)] = acc
                    nc.vector.tensor_copy(acc[:, 0:256], p1[:])
                    nc.vector.tensor_copy(acc[:, 256 : D + 2], p2[:])
                else:
                    acc = accs[(x, qb, qt_i)]
                    nc.vector.tensor_tensor(
                        acc[:, 0:256], acc[:, 0:256], p1[:], ALU.add
                    )
                    nc.vector.tensor_tensor(
                        acc[:, 256 : D + 2], acc[:, 256 : D + 2], p2[:], ALU.add
                    )
                if r == R - 1:
                    # finalize: out = acc[:, :512] / acc[:, 512] + bv.
                    # mult on Act (scale=rr), bv-add on DVE: splits the tail
                    # chain across two engines; out DMAs alternate rings.
                    rr = attnp.tile([128, 1], dt.float32, tag="rr", bufs=4, name="rr")
                    nc.vector.reciprocal(rr[:], acc[:, D : D + 1])
                    ot = attnp.tile([128, D], dt.float32, tag="ot", bufs=3, name="ot")
                    nc.vector.scalar_tensor_tensor(
                        ot[:], acc[:, 0:D], rr[:], bv[:], ALU.mult, ALU.add
                    )
                    r0 = qb * 512 + qt_i * 128
                    eng = nc.gpsimd if qt_i % 2 == 0 else nc.sync
                    eng.dma_start(
                        out_d.ap()[r0 : r0 + 128, col * D : (col + 1) * D], ot[:]
                    )

        for x, col in (("b", 1), ("a", 0)):
            if x == "a":
                q_proj("a")
            for r in range(R):
                if (x, r) in pre:
                    ktile, vtile = pre[(x, r)]
                else:
                    ktile, vtile = load_shard(attnp, x, r)
                for qb in range(NB):
                    exps = do_scores(x, r, qb, ktile)
                    pending.append((x, col, r, qb, exps, vtile))
                    if len(pending) > 1:
                        do_attnv(*pending.pop(0))
        while pending:
            do_attnv(*pending.pop(0))
        attnp.release()
        projp.release()
        wkvp.release()
        qtp.release()
        const.release()
        dram.release()
        psum.release()

    nc.compile()
    return nc


_NC = None


def _get_nc():
    global _NC
    if _NC is None:
        _NC = build()
    return _NC


def _chunk_w(w):
    """[X, Y] -> [128, X//128, Y] partition-chunked, contiguous."""
    x, y = w.shape
    return np.ascontiguousarray(w.reshape(x // 128, 128, y).transpose(1, 0, 2))


def _chunk_b(b):
    return np.ascontiguousarray(np.asarray(b, np.float32).reshape(-1, 128).T)


def prep_in_maps(za, zb, W1, b1, W2, b2, Wq, bq, Wk, bk, Wv, bv):
    za = np.asarray(za, np.float32)
    zb = np.asarray(zb, np.float32)
    W1 = np.asarray(W1, np.float32)
    W2 = np.asarray(W2, np.float32)
    Wq = np.asarray(Wq, np.float32)
    Wk = np.asarray(Wk, np.float32)
    Wv = np.asarray(Wv, np.float32)
    b1 = np.asarray(b1, np.float32)
    b2 = np.asarray(b2, np.float32)
    bq = np.asarray(bq, np.float32)
    bk = np.asarray(bk, np.float32)
    bv = np.asarray(bv, np.float32)

    f8 = dt.np(F8)
    bf = dt.np(BF16)
    # Wk folded into the query side; 1/SCALE split as 1/QS per fp8 operand.
    Wqk = (Wq @ Wk.T) * (QS / SCALE)
    bqk = (bq @ Wk.T) * (QS / SCALE)
    shared = {
        "W1t8": np.ascontiguousarray(_chunk_w(W1 * 16.0)[:, : HC // 2]).astype(f8),
        "W1tb": np.ascontiguousarray(_chunk_w(W1 * 16.0)[:, HC // 2 :]).astype(bf),
        "W2t": _chunk_w(W2),
        "Wqkt": _chunk_w(Wqk).astype(bf),
        "Wvt": _chunk_w(Wv).astype(bf),
        "b1t": _chunk_b(b1),
        "b2t": _chunk_b(b2 - W2.sum(axis=0)),
        "bqkt": _chunk_b(bqk),
        "bvt": np.ascontiguousarray(np.broadcast_to(bv, (128, D)).astype(np.float32)),
        "vpad": np.ascontiguousarray(
            np.broadcast_to(
                np.tile(np.array([1.0, 0.0], np.float32), KC), (128, 2 * KC)
            )
        ).astype(f8),
    }
    zaT = np.ascontiguousarray(za.T)  # [H, N]
    zbT = np.ascontiguousarray(zb.T)
    in_maps = []
    for c in range(R):
        cs = slice(c * NL, (c + 1) * NL)
        in_maps.append(
            {
                "zaT8": np.ascontiguousarray(_chunk_w(zaT[:, cs])[:, : HC // 2]).astype(f8),
                "zaTb": np.ascontiguousarray(_chunk_w(zaT[:, cs])[:, HC // 2 :]).astype(bf),
                "zbT8": np.ascontiguousarray(_chunk_w(zbT[:, cs])[:, : HC // 2]).astype(f8),
                "zbTb": np.ascontiguousarray(_chunk_w(zbT[:, cs])[:, HC // 2 :]).astype(bf),
                **shared,
            }
        )
    return in_maps


def kernel(**inputs) -> np.ndarray:
    nc = _get_nc()
    in_maps = prep_in_maps(**inputs)
    res = run_bass_kernel_spmd(nc, in_maps, core_ids=list(range(R)))
    return np.concatenate([res.results[c]["out"] for c in range(R)], axis=0)


# revision 30
# speedup vs baseline: 1.0117x; 1.0117x over previous
"""Trainium2 8-core kernel for the Contrast module (fp8 DoubleRow + split AG).

    za_p = ELU(za @ W1 + b1) @ W2 + b2          (same for zb)
    za_ca = softmax((za_p Wq + bq)(zb_p Wk + bk)^T / sqrt(256)) @ (zb_p Wv + bv)
    zb_ca = softmax((zb_p Wq + bq)(za_p Wk + bk)^T / sqrt(256)) @ (za_p Wv + bv)
    out = concat(za_ca, zb_ca, axis=1)

Key structural choices:
 - Wk is folded into the query side on the host: with Wqk = Wq Wk^T / s and
   bqk = bq Wk^T / s, softmax(Q K^T/s) == softmax((P_q Wqk + bqk) P_k^T)
   (the dropped (P_q Wq + bq)·bk^T term is constant along k, so it cancels
   in softmax). This removes the entire K projection: the score matmul
   contracts Q'' (fp8) against the raw projections P_k (fp8).
 - fp8 scaling: Wqk carries 1/4 and the P_k fp8 copy carries 1/4 (product
   1/16 = 1/sqrt(D/2)); splitting the scale keeps both operands out of the
   fp8e4 subnormal range.
 - TWO AllGathers, one per attention direction, each fully overlapped:
   AG(dir b) ships P_a/V_a during the zb projection; AG(dir a) ships
   P_b/V_b during direction-b attention. Neither exposes collective time
   on the PE critical path (the single-AG variant stalled PE ~80us).
 - Attention matmuls (scores and attn@V) run in fp8 DoubleRow perf mode;
   PSUM accumulation stays f32.
 - Projections (W1/ELU/W2, V, Q'') stay f32r for accuracy.

Sharding: rows data-parallel across 8 cores; weights replicated.
Layout notes:
 - Activations flow feature-major ("transposed"): zaT [h, n] so every
   matmul contracts over the partition axis without on-chip transposes.
 - softmax denominator: V panels carry a ones column; attn@V is split into
   N=256 and N=258 matmuls so the rowsum accumulates in PSUM col 512.
 - No max-subtraction in softmax: scores are ~N(0, 0.85^2); exp(s-2) keeps
   fp8 exp outputs in range, and the e^-2 cancels against the denominator.
 - ELU+1 = max(x+1, min(exp(x), 1)) via one Act + two DVE ops
   (scalar_tensor_tensor fuses the min+max); the -1 is folded into b2.
 - bv is added at finalize (softmax weights sum to 1).
"""

import numpy as np

import concourse.mybir as mybir
import concourse.tile as tile
from concourse import bacc
from concourse.bass_utils import run_bass_kernel_spmd

dt = mybir.dt
AF = mybir.ActivationFunctionType
ALU = mybir.AluOpType
PM = mybir.MatmulPerfMode

R = 8            # cores
N = 8192         # total rows
H = 1024         # hidden
D = 512          # attention dim
NL = N // R      # rows per core
HC = H // 128    # 8 h-chunks
DC = D // 128    # 4 d-chunks
NB = NL // 512   # 2 n-blocks per core slice
KC = NL // 128   # 8 key-chunks per shard
SCALE = 16.0     # sqrt(512/2)
QS = 4.0         # per-operand fp8 scale split: (Q''/4)·(P/4) = Q''P/16
KVF = D * NL     # elements per P (or V) shard section
SHARD = 2 * KVF  # P_x | V_x, fp8 bytes per core per direction

F32R = dt.float32r
BF16 = dt.bfloat16
F8 = dt.float8e4


def _r(ap):
    return ap.bitcast(F32R)


def build():
    nc = bacc.Bacc("TRN2", target_bir_lowering=False, debug=False, num_devices=R)

    def inp(name, shape, dtype=dt.float32):
        return nc.dram_tensor(name, shape, dtype, kind="ExternalInput")

    zT8 = {
        "a": inp("zaT8", [128, HC // 2, NL], F8),
        "b": inp("zbT8", [128, HC // 2, NL], F8),
    }
    zTb = {
        "a": inp("zaTb", [128, HC // 2, NL], BF16),
        "b": inp("zbTb", [128, HC // 2, NL], BF16),
    }
    w18 = inp("W1t8", [128, HC // 2, H], F8)
    w1b = inp("W1tb", [128, HC // 2, H], BF16)
    w2 = inp("W2t", [128, HC, D])
    wqk = inp("Wqkt", [128, DC, D], BF16)
    wv = inp("Wvt", [128, DC, D], BF16)
    b1d = inp("b1t", [128, HC])
    b2d = inp("b2t", [128, DC])
    bqkd = inp("bqkt", [128, DC])
    bvd = inp("bvt", [128, D])
    vpadd = inp("vpad", [128, 2 * KC], F8)
    out_d = nc.dram_tensor("out", [NL, 2 * D], dt.float32, kind="ExternalOutput")

    with tile.TileContext(nc) as tc:
        psum = tc.alloc_tile_pool(name="psum", bufs=1, space="PSUM")
        dram = tc.alloc_tile_pool(name="dram", bufs=1, space="DRAM")
        const = tc.alloc_tile_pool(name="const", bufs=1)
        qtp = tc.alloc_tile_pool(name="qtp", bufs=1)
        wkvp = tc.alloc_tile_pool(name="wkvp", bufs=1)
        projp = tc.alloc_tile_pool(name="projp", bufs=1)

        # ---- DMAs ordered by first PE use: (w1t[hc], z[hc]) pairs so the
        # hc-accumulation of the first W1 block starts as soon as each pair
        # lands; biases next (first ELU needs them only ~2us in).
        w1t8 = projp.tile([128, HC // 2, H], F8, name="w1t8")
        w1tb = projp.tile([128, HC // 2, H], BF16, name="w1tb")
        zt = {}
        b1 = const.tile([128, HC], dt.float32, name="b1")
        b2 = const.tile([128, DC], dt.float32, name="b2")
        bqk = const.tile([128, DC], dt.float32, name="bqk")
        bv = const.tile([128, D], dt.float32, name="bv")
        z0_8 = projp.tile([128, HC // 2, 512], F8, tag="z8", bufs=2, name="z8_a0")
        z0_b = projp.tile([128, HC // 2, 512], BF16, tag="zb", bufs=2, name="zb_a0")
        zt[("a", 0)] = (z0_8, z0_b)
        for hc in range(HC // 2):
            nc.sync.dma_start(w1t8[:, hc, :], w18.ap()[:, hc, :])
            nc.scalar.dma_start(w1tb[:, hc, :], w1b.ap()[:, hc, :])
            nc.gpsimd.dma_start(z0_8[:, hc, :], zT8["a"].ap()[:, hc, 0:512])
            nc.gpsimd.dma_start(z0_b[:, hc, :], zTb["a"].ap()[:, hc, 0:512])
        nc.sync.dma_start(b1[:], b1d.ap())
        z1_8 = projp.tile([128, HC // 2, 512], F8, tag="z8", bufs=2, name="z8_a1")
        z1_b = projp.tile([128, HC // 2, 512], BF16, tag="zb", bufs=2, name="zb_a1")
        zt[("a", 1)] = (z1_8, z1_b)
        for hc in range(HC // 2):
            nc.gpsimd.dma_start(z1_8[:, hc, :], zT8["a"].ap()[:, hc, 512:1024])
            nc.gpsimd.dma_start(z1_b[:, hc, :], zTb["a"].ap()[:, hc, 512:1024])
        w2t = projp.tile([128, HC, D], F32R, name="w2t")
        nc.sync.dma_start(b2[:], b2d.ap())
        wqkt = wkvp.tile([128, DC, D], BF16, name="wqkt")
        wvt = wkvp.tile([128, DC, D], BF16, name="wvt")
        # w2t/wvt/wqkt ride the gpsimd ring BEHIND the z chunks so their
        # 3MB doesn't compete with the startup-critical W1/z feed.
        nc.gpsimd.dma_start(w2t[:], _r(w2.ap()))
        nc.gpsimd.dma_start(wvt[:], wv.ap())
        nc.gpsimd.dma_start(wqkt[:], wqk.ap())
        nc.sync.dma_start(bqk[:], bqkd.ap())
        nc.sync.dma_start(bv[:], bvd.ap())
        # exp(x - 2): keeps fp8 exp outputs in range for scores up to ~7.4;
        # the e^-2 cancels between numerator and the ones-column denominator.
        negc = const.tile([128, 1], dt.float32, name="negc")
        nc.vector.memset(negc[:], -2.0)
        # z(b) chunk DMAs issued up front on the gpsimd ring; the z-tag
        # rotation (bufs=2) gates each write on the a-block consumers, so
        # they stream in as soon as the W1(a) matmuls retire.
        for nb in range(NB):
            zb8 = projp.tile([128, HC // 2, 512], F8, tag="z8", bufs=2, name=f"z8_b{nb}")
            zbb = projp.tile([128, HC // 2, 512], BF16, tag="zb", bufs=2, name=f"zb_b{nb}")
            zt[("b", nb)] = (zb8, zbb)
            for hc in range(HC // 2):
                nc.gpsimd.dma_start(
                    zb8[:, hc, :], zT8["b"].ap()[:, hc, nb * 512 : (nb + 1) * 512]
                )
                nc.gpsimd.dma_start(
                    zbb[:, hc, :], zTb["b"].ap()[:, hc, nb * 512 : (nb + 1) * 512]
                )

        pT = {
            "a": wkvp.tile([128, DC, NL], BF16, name="pta"),
            "b": wkvp.tile([128, DC, NL], BF16, name="ptb"),
        }

        # per-direction fused AG buffers: [P_x | V_x] fp8
        agin = {x: dram.tile([SHARD], F8, name=f"agin_{x}") for x in ("b", "a")}
        agout = {
            x: dram.tile([R * SHARD], F8, name=f"agout_{x}", addr_space="Shared")
            for x in ("b", "a")
        }

        # ================= projection + P/V shards =================
        def mmtile(g=None):
            """single-bank accumulation tile; 5 rotating buffers so the PE
            never waits on a lagging Act/DVE consumer."""
            return psum.tile([128, 512], dt.float32, tag="mm", bufs=4, name="ps1")

        for src, other in (("a", "b"), ("b", "a")):
            for nb in range(NB):
                ns = slice(nb * 512, (nb + 1) * 512)
                z8, zb16 = zt[(src, nb)]
                hT = projp.tile([128, HC, 512], F32R, tag="h", bufs=2, name=f"h_{src}{nb}")
                # ELU(x)+1 = max(x+1, min(exp(x), 1)), x = ps + b1.
                # The combine (stt) is software-pipelined one block behind so
                # the DVE's PSUM reader (xp1) is never queued behind a combine
                # and the PSUM slot frees right after Act's Exp + DVE's add.
                stt_q = []
                for d1c in range(HC):
                    ps = mmtile()
                    for c in range(HC // 4):
                        nc.tensor.matmul(
                            ps,
                            w1t8[:, 2 * c : 2 * c + 2, d1c * 128 : (d1c + 1) * 128],
                            z8[:, 2 * c : 2 * c + 2, :],
                            start=(c == 0),
                            stop=False,
                            perf_mode=PM.DoubleRow,
                        )
                    for hc in range(HC // 2):
                        nc.tensor.matmul(
                            ps,
                            w1tb[:, hc, d1c * 128 : (d1c + 1) * 128],
                            zb16[:, hc, :],
                            start=False,
                            stop=(hc == HC // 2 - 1),
                        )
                    e = projp.tile([128, 512], dt.float32, tag="e", bufs=3, name="e")
                    xp1 = projp.tile([128, 512], dt.float32, tag="xp1", bufs=3, name="xp1")
                    # W1 shipped x16 in fp8; fold the 1/16 into the Act scale
                    nc.scalar.activation(
                        e[:], ps, AF.Exp, bias=b1[:, d1c : d1c + 1], scale=1.0 / 16
                    )
                    nc.scalar.activation(
                        xp1[:], ps, AF.Relu, bias=b1[:, d1c : d1c + 1], scale=1.0 / 16
                    )
                    stt_q.append((d1c, e, xp1))
                    if len(stt_q) > 1:
                        dd, ee, xx = stt_q.pop(0)
                        nc.vector.scalar_tensor_tensor(
                            hT[:, dd, :], ee[:], 1.0, xx[:], ALU.min, ALU.add
                        )
                dd, ee, xx = stt_q.pop(0)
                nc.vector.scalar_tensor_tensor(
                    hT[:, dd, :], ee[:], 1.0, xx[:], ALU.min, ALU.add
                )
                for d2c in range(DC):
                    ps = mmtile()
                    for d1c in range(HC):
                        nc.tensor.matmul(
                            ps,
                            w2t[:, d1c, d2c * 128 : (d2c + 1) * 128],
                            hT[:, d1c, :],
                            start=(d1c == 0),
                            stop=(d1c == HC - 1),
                        )
                    nc.scalar.activation(
                        pT[src][:, d2c, ns], ps, AF.Identity, bias=b2[:, d2c : d2c + 1]
                    )

            # stage this src's P (fp8, /4) + V into the *other* direction's
            # AG shard, then fire that direction's AllGather.
            pv = agin[other][0:KVF].rearrange("(d n) -> d n", n=NL)
            vv = agin[other][KVF : 2 * KVF].rearrange("(n d) -> n d", d=D)
            for dc in range(DC):
                for nb in range(NB):
                    ns = slice(nb * 512, (nb + 1) * 512)
                    s = projp.tile([128, 512], F8, tag="stg", bufs=16, name="stg_p")
                    nc.vector.tensor_scalar(
                        s[:], pT[src][:, dc, ns], 1.0 / QS, None, ALU.mult
                    )
                    nc.sync.dma_start(pv[dc * 128 : (dc + 1) * 128, ns], s[:])
            for nt in range(KC):
                ps = mmtile()
                for d2c in range(DC):
                    nc.tensor.matmul(
                        ps,
                        pT[src][:, d2c, nt * 128 : (nt + 1) * 128],
                        wvt[:, d2c, :],
                        start=(d2c == 0),
                        stop=(d2c == DC - 1),
                    )
                s = projp.tile([128, 512], F8, tag="stg", bufs=16, name="stg_v")
                nc.scalar.activation(s[:], ps, AF.Copy)
                nc.sync.dma_start(vv[nt * 128 : (nt + 1) * 128, :], s[:])
            nc.gpsimd.collective_compute(
                "AllGather",
                ALU.bypass,
                ins=[agin[other].opt()],
                outs=[agout[other].opt()],
                replica_groups=[list(range(R))],
            )

        # Pre-load direction-b shards r=0,1 into FRESH SBUF (attnp pool
        # created while projp is still live, so these tiles do not overlap
        # released projection space — otherwise their DMAs inherit an
        # anti-dependency on the last Q'' matmul and attention starts late).
        attnp = tc.alloc_tile_pool(name="attnp", bufs=1)
        pre = {}

        def load_shard(pool, x, r, eng=None):
            eng = eng or nc.sync
            base = r * SHARD
            ktile = pool.tile([128, DC, NL], F8, tag="kt", bufs=2, name=f"kt{r}")
            eng.dma_start(
                ktile[:],
                agout[x][base : base + KVF].rearrange(
                    "(dc p n) -> p dc n", p=128, n=NL
                ),
            )
            vtile = pool.tile([128, KC, D + 2], F8, tag="vt", bufs=3, name=f"vt{r}")
            eng.dma_start(
                vtile[:, :, 0:D],
                agout[x][base + KVF : base + 2 * KVF].rearrange(
                    "(kc p d) -> p kc d", p=128, d=D
                ),
            )
            eng.dma_start(
                vtile[:, :, D : D + 2],
                vpadd.ap().rearrange("p (kc c) -> p kc c", c=2),
            )
            return ktile, vtile

        for r in (0, 1):
            pre[("b", r)] = load_shard(attnp, "b", r, eng=nc.gpsimd)

        # ================= queries Q'' =================
        # Q''(b) is needed right away; Q''(a) is deferred until just before
        # direction-a attention so direction-b attention starts ~8us sooner.
        qT = {}

        def q_proj(x):
            qT[x] = qtp.tile([128, DC, NL], F8, name=f"qt_{x}")
            for dc in range(DC):
                for nb in range(NB):
                    ps = mmtile()
                    for d2c in range(DC):
                        nc.tensor.matmul(
                            ps,
                            wqkt[:, d2c, dc * 128 : (dc + 1) * 128],
                            pT[x][:, d2c, nb * 512 : (nb + 1) * 512],
                            start=(d2c == 0),
                            stop=(d2c == DC - 1),
                        )
                    nc.scalar.activation(
                        qT[x][:, dc, nb * 512 : (nb + 1) * 512],
                        ps,
                        AF.Identity,
                        bias=bqk[:, dc : dc + 1],
                    )

        q_proj("b")
        q_proj("a")

        # ================= attention (fp8 DoubleRow) =================
        # Software-pipelined: block i+1's score matmuls are issued before
        # block i's attn@V so the Exp activations never wait on PE.
        accs = {}
        pending = []  # deferred attn@V closures, one per (x, r, qb) block

        def do_scores(x, r, qb, ktile):
            qs = slice(qb * 512, (qb + 1) * 512)
            exps = []
            for kp in range(KC // 2):
                ex = attnp.tile([128, 2, 512], F8, tag="exp", bufs=16, name="ex")
                for j in range(2):
                    kt_i = 2 * kp + j
                    ps = mmtile()
                    for c in range(DC // 2):
                        nc.tensor.matmul(
                            ps,
                            ktile[:, 2 * c : 2 * c + 2,
                                  kt_i * 128 : (kt_i + 1) * 128],
                            qT[x][:, 2 * c : 2 * c + 2, qs],
                            start=(c == 0),
                            stop=(c == DC // 2 - 1),
                            perf_mode=PM.DoubleRow,
                        )
                    nc.scalar.activation(ex[:, j, :], ps, AF.Exp, bias=negc[:])
                exps.append(ex)
            return exps

        def do_attnv(x, col, r, qb, exps, vtile):
            for qt_i in range(4):
                qsl = slice(qt_i * 128, (qt_i + 1) * 128)
                p1 = psum.tile([128, 256], dt.float32, tag="po1", bufs=2, name="po1")
                p2 = psum.tile([128, 258], dt.float32, tag="po2", bufs=2, name="po2")
                for kp in range(KC // 2):
                    nc.tensor.matmul(
                        p1[:],
                        exps[kp][:, :, qsl],
                        vtile[:, 2 * kp : 2 * kp + 2, 0:256],
                        start=(kp == 0),
                        stop=(kp == KC // 2 - 1),
                        perf_mode=PM.DoubleRow,
                    )
                    nc.tensor.matmul(
                        p2[:],
                        exps[kp][:, :, qsl],
                        vtile[:, 2 * kp : 2 * kp + 2, 256 : D + 2],
                        start=(kp == 0),
                        stop=(kp == KC // 2 - 1),
                        perf_mode=PM.DoubleRow,
                    )
                if r == 0:
                    acc = attnp.tile(
                        [128, D + 2], dt.float32, tag="acc", bufs=8,
                        name=f"acc{qb}{qt_i}",
                    )
                    accs[(x, qb, qt_i)] = acc
                    nc.vector.tensor_copy(acc[:, 0:256], p1[:])
                    nc.vector.tensor_copy(acc[:, 256 : D + 2], p2[:])
                else:
                    acc = accs[(x, qb, qt_i)]
                    nc.vector.tensor_tensor(
                        acc[:, 0:256], acc[:, 0:256], p1[:], ALU.add
                    )
                    nc.vector.tensor_tensor(
                        acc[:, 256 : D + 2], acc[:, 256 : D + 2], p2[:], ALU.add
                    )
                if r == R - 1:
                    # finalize: out = acc[:, :512] / acc[:, 512] + bv.
                    # mult on Act (scale=rr), bv-add on DVE: splits the tail
                    # chain across two engines; out DMAs alternate rings.
                    rr = attnp.tile([128, 1], dt.float32, tag="rr", bufs=4, name="rr")
                    nc.vector.reciprocal(rr[:], acc[:, D : D + 1])
                    ot = attnp.tile([128, D], dt.float32, tag="ot", bufs=3, name="ot")
                    nc.vector.scalar_tensor_tensor(
                        ot[:], acc[:, 0:D], rr[:], bv[:], ALU.mult, ALU.add
                    )
                    r0 = qb * 512 + qt_i * 128
                    eng = nc.gpsimd if qt_i % 2 == 0 else nc.sync
                    eng.dma_start(
                        out_d.ap()[r0 : r0 + 128, col * D : (col + 1) * D], ot[:]
                    )

        for x, col in (("b", 1), ("a", 0)):
            for r in range(R):
                if (x, r) in pre:
                    ktile, vtile = pre[(x, r)]
                else:
                    ktile, vtile = load_shard(attnp, x, r)
                for qb in range(NB):
                    exps = do_scores(x, r, qb, ktile)
                    pending.append((x, col, r, qb, exps, vtile))
                    if len(pending) > 1:
                        do_attnv(*pending.pop(0))
        while pending:
            do_attnv(*pending.pop(0))
        attnp.release()
        projp.release()
        wkvp.release()
        qtp.release()
        const.release()
        dram.release()
        psum.release()

    nc.compile()
    return nc


_NC = None


def _get_nc():
    global _NC
    if _NC is None:
        _NC = build()
    return _NC


def _chunk_w(w):
    """[X, Y] -> [128, X//128, Y] partition-chunked, contiguous."""
    x, y = w.shape
    return np.ascontiguousarray(w.reshape(x // 128, 128, y).transpose(1, 0, 2))


def _chunk_b(b):
    return np.ascontiguousarray(np.asarray(b, np.float32).reshape(-1, 128).T)


def prep_in_maps(za, zb, W1, b1, W2, b2, Wq, bq, Wk, bk, Wv, bv):
    za = np.asarray(za, np.float32)
    zb = np.asarray(zb, np.float32)
    W1 = np.asarray(W1, np.float32)
    W2 = np.asarray(W2, np.float32)
    Wq = np.asarray(Wq, np.float32)
    Wk = np.asarray(Wk, np.float32)
    Wv = np.asarray(Wv, np.float32)
    b1 = np.asarray(b1, np.float32)
    b2 = np.asarray(b2, np.float32)
    bq = np.asarray(bq, np.float32)
    bk = np.asarray(bk, np.float32)
    bv = np.asarray(bv, np.float32)

    f8 = dt.np(F8)
    bf = dt.np(BF16)
    # Wk folded into the query side; 1/SCALE split as 1/QS per fp8 operand.
    Wqk = (Wq @ Wk.T) * (QS / SCALE)
    bqk = (bq @ Wk.T) * (QS / SCALE)
    shared = {
        "W1t8": np.ascontiguousarray(_chunk_w(W1 * 16.0)[:, : HC // 2]).astype(f8),
        "W1tb": np.ascontiguousarray(_chunk_w(W1 * 16.0)[:, HC // 2 :]).astype(bf),
        "W2t": _chunk_w(W2),
        "Wqkt": _chunk_w(Wqk).astype(bf),
        "Wvt": _chunk_w(Wv).astype(bf),
        "b1t": _chunk_b(b1),
        "b2t": _chunk_b(b2 - W2.sum(axis=0)),
        "bqkt": _chunk_b(bqk),
        "bvt": np.ascontiguousarray(np.broadcast_to(bv, (128, D)).astype(np.float32)),
        "vpad": np.ascontiguousarray(
            np.broadcast_to(
                np.tile(np.array([1.0, 0.0], np.float32), KC), (128, 2 * KC)
            )
        ).astype(f8),
    }
    zaT = np.ascontiguousarray(za.T)  # [H, N]
    zbT = np.ascontiguousarray(zb.T)
    in_maps = []
    for c in range(R):
        cs = slice(c * NL, (c + 1) * NL)
        in_maps.append(
            {
                "zaT8": np.ascontiguousarray(_chunk_w(zaT[:, cs])[:, : HC // 2]).astype(f8),
                "zaTb": np.ascontiguousarray(_chunk_w(zaT[:, cs])[:, HC // 2 :]).astype(bf),
                "zbT8": np.ascontiguousarray(_chunk_w(zbT[:, cs])[:, : HC // 2]).astype(f8),
                "zbTb": np.ascontiguousarray(_chunk_w(zbT[:, cs])[:, HC // 2 :]).astype(bf),
                **shared,
            }
        )
    return in_maps


def kernel(**inputs) -> np.ndarray:
    nc = _get_nc()
    in_maps = prep_in_maps(**inputs)
    res = run_bass_kernel_spmd(nc, in_maps, core_ids=list(range(R)))
    return np.concatenate([res.results[c]["out"] for c in range(R)], axis=0)


# revision 31
# speedup vs baseline: 1.0124x; 1.0008x over previous
"""Trainium2 8-core kernel for the Contrast module (fp8 DoubleRow + split AG).

    za_p = ELU(za @ W1 + b1) @ W2 + b2          (same for zb)
    za_ca = softmax((za_p Wq + bq)(zb_p Wk + bk)^T / sqrt(256)) @ (zb_p Wv + bv)
    zb_ca = softmax((zb_p Wq + bq)(za_p Wk + bk)^T / sqrt(256)) @ (za_p Wv + bv)
    out = concat(za_ca, zb_ca, axis=1)

Key structural choices:
 - Wk is folded into the query side on the host: with Wqk = Wq Wk^T / s and
   bqk = bq Wk^T / s, softmax(Q K^T/s) == softmax((P_q Wqk + bqk) P_k^T)
   (the dropped (P_q Wq + bq)·bk^T term is constant along k, so it cancels
   in softmax). This removes the entire K projection: the score matmul
   contracts Q'' (fp8) against the raw projections P_k (fp8).
 - fp8 scaling: Wqk carries 1/4 and the P_k fp8 copy carries 1/4 (product
   1/16 = 1/sqrt(D/2)); splitting the scale keeps both operands out of the
   fp8e4 subnormal range.
 - TWO AllGathers, one per attention direction, each fully overlapped:
   AG(dir b) ships P_a/V_a during the zb projection; AG(dir a) ships
   P_b/V_b during direction-b attention. Neither exposes collective time
   on the PE critical path (the single-AG variant stalled PE ~80us).
 - Attention matmuls (scores and attn@V) run in fp8 DoubleRow perf mode;
   PSUM accumulation stays f32.
 - W1 is split by contraction half: hidden dims 0..511 in fp8 DoubleRow
   (weights shipped x16; the 1/16 folds into the Act scale) and 512..1023
   in bf16, balancing PE time against the 2e-2 error budget. W2 stays
   f32r; V and Q'' run in bf16.

Sharding: rows data-parallel across 8 cores; weights replicated.
Layout notes:
 - Activations flow feature-major ("transposed"): zaT [h, n] so every
   matmul contracts over the partition axis without on-chip transposes.
 - softmax denominator: V panels carry a ones column; attn@V is split into
   N=256 and N=258 matmuls so the rowsum accumulates in PSUM col 512.
 - No max-subtraction in softmax: scores are ~N(0, 0.85^2); exp(s-2) keeps
   fp8 exp outputs in range, and the e^-2 cancels against the denominator.
 - ELU+1 = relu(x) + min(exp(x), 1): both PSUM readers (Exp, Relu) live
   on the Act engine and one DVE scalar_tensor_tensor combines them; the
   -1 is folded into b2.
 - Single-bank [128,512] PSUM tiles with 4 rotating buffers keep the PE
   ahead of the Act/DVE consumers (pairing two groups per tile stalled
   the PE ~1.5us per pair).
 - DMA triggers are spread across the sync/scalar/gpsimd queue rings by
   phase so no blocking trigger ever sits ahead of live work.
 - bv is added at finalize (softmax weights sum to 1).
"""

import numpy as np

import concourse.mybir as mybir
import concourse.tile as tile
from concourse import bacc
from concourse.bass_utils import run_bass_kernel_spmd

dt = mybir.dt
AF = mybir.ActivationFunctionType
ALU = mybir.AluOpType
PM = mybir.MatmulPerfMode

R = 8            # cores
N = 8192         # total rows
H = 1024         # hidden
D = 512          # attention dim
NL = N // R      # rows per core
HC = H // 128    # 8 h-chunks
DC = D // 128    # 4 d-chunks
NB = NL // 512   # 2 n-blocks per core slice
KC = NL // 128   # 8 key-chunks per shard
SCALE = 16.0     # sqrt(512/2)
QS = 4.0         # per-operand fp8 scale split: (Q''/4)·(P/4) = Q''P/16
KVF = D * NL     # elements per P (or V) shard section
SHARD = 2 * KVF  # P_x | V_x, fp8 bytes per core per direction

F32R = dt.float32r
BF16 = dt.bfloat16
F8 = dt.float8e4


def _r(ap):
    return ap.bitcast(F32R)


def build():
    nc = bacc.Bacc("TRN2", target_bir_lowering=False, debug=False, num_devices=R)

    def inp(name, shape, dtype=dt.float32):
        return nc.dram_tensor(name, shape, dtype, kind="ExternalInput")

    zT8 = {
        "a": inp("zaT8", [128, HC // 2, NL], F8),
        "b": inp("zbT8", [128, HC // 2, NL], F8),
    }
    zTb = {
        "a": inp("zaTb", [128, HC // 2, NL], BF16),
        "b": inp("zbTb", [128, HC // 2, NL], BF16),
    }
    w18 = inp("W1t8", [128, HC // 2, H], F8)
    w1b = inp("W1tb", [128, HC // 2, H], BF16)
    w2 = inp("W2t", [128, HC, D])
    wqk = inp("Wqkt", [128, DC, D], BF16)
    wv = inp("Wvt", [128, DC, D], BF16)
    b1d = inp("b1t", [128, HC])
    b2d = inp("b2t", [128, DC])
    bqkd = inp("bqkt", [128, DC])
    bvd = inp("bvt", [128, D])
    vpadd = inp("vpad", [128, 2 * KC], F8)
    out_d = nc.dram_tensor("out", [NL, 2 * D], dt.float32, kind="ExternalOutput")

    with tile.TileContext(nc) as tc:
        psum = tc.alloc_tile_pool(name="psum", bufs=1, space="PSUM")
        dram = tc.alloc_tile_pool(name="dram", bufs=1, space="DRAM")
        const = tc.alloc_tile_pool(name="const", bufs=1)
        qtp = tc.alloc_tile_pool(name="qtp", bufs=1)
        wkvp = tc.alloc_tile_pool(name="wkvp", bufs=1)
        projp = tc.alloc_tile_pool(name="projp", bufs=1)

        # ---- DMAs ordered by first PE use: (w1t[hc], z[hc]) pairs so the
        # hc-accumulation of the first W1 block starts as soon as each pair
        # lands; biases next (first ELU needs them only ~2us in).
        w1t8 = projp.tile([128, HC // 2, H], F8, name="w1t8")
        w1tb = projp.tile([128, HC // 2, H], BF16, name="w1tb")
        zt = {}
        b1 = const.tile([128, HC], dt.float32, name="b1")
        b2 = const.tile([128, DC], dt.float32, name="b2")
        bqk = const.tile([128, DC], dt.float32, name="bqk")
        bv = const.tile([128, D], dt.float32, name="bv")
        z0_8 = projp.tile([128, HC // 2, 512], F8, tag="z8", bufs=2, name="z8_a0")
        z0_b = projp.tile([128, HC // 2, 512], BF16, tag="zb", bufs=2, name="zb_a0")
        zt[("a", 0)] = (z0_8, z0_b)
        for hc in range(HC // 2):
            nc.sync.dma_start(w1t8[:, hc, :], w18.ap()[:, hc, :])
            nc.scalar.dma_start(w1tb[:, hc, :], w1b.ap()[:, hc, :])
            nc.gpsimd.dma_start(z0_8[:, hc, :], zT8["a"].ap()[:, hc, 0:512])
            nc.gpsimd.dma_start(z0_b[:, hc, :], zTb["a"].ap()[:, hc, 0:512])
        nc.sync.dma_start(b1[:], b1d.ap())
        z1_8 = projp.tile([128, HC // 2, 512], F8, tag="z8", bufs=2, name="z8_a1")
        z1_b = projp.tile([128, HC // 2, 512], BF16, tag="zb", bufs=2, name="zb_a1")
        zt[("a", 1)] = (z1_8, z1_b)
        for hc in range(HC // 2):
            nc.gpsimd.dma_start(z1_8[:, hc, :], zT8["a"].ap()[:, hc, 512:1024])
            nc.gpsimd.dma_start(z1_b[:, hc, :], zTb["a"].ap()[:, hc, 512:1024])
        w2t = projp.tile([128, HC, D], F32R, name="w2t")
        nc.sync.dma_start(b2[:], b2d.ap())
        wqkt = wkvp.tile([128, DC, D], BF16, name="wqkt")
        wvt = wkvp.tile([128, DC, D], BF16, name="wvt")
        # w2t/wvt/wqkt ride the gpsimd ring BEHIND the z chunks so their
        # 3MB doesn't compete with the startup-critical W1/z feed.
        nc.gpsimd.dma_start(w2t[:], _r(w2.ap()))
        nc.gpsimd.dma_start(wvt[:], wv.ap())
        nc.gpsimd.dma_start(wqkt[:], wqk.ap())
        nc.sync.dma_start(bqk[:], bqkd.ap())
        nc.sync.dma_start(bv[:], bvd.ap())
        # exp(x - 2): keeps fp8 exp outputs in range for scores up to ~7.4;
        # the e^-2 cancels between numerator and the ones-column denominator.
        negc = const.tile([128, 1], dt.float32, name="negc")
        nc.vector.memset(negc[:], -2.0)
        # z(b) chunk DMAs issued up front on the gpsimd ring; the z-tag
        # rotation (bufs=2) gates each write on the a-block consumers, so
        # they stream in as soon as the W1(a) matmuls retire.
        for nb in range(NB):
            zb8 = projp.tile([128, HC // 2, 512], F8, tag="z8", bufs=2, name=f"z8_b{nb}")
            zbb = projp.tile([128, HC // 2, 512], BF16, tag="zb", bufs=2, name=f"zb_b{nb}")
            zt[("b", nb)] = (zb8, zbb)
            for hc in range(HC // 2):
                nc.gpsimd.dma_start(
                    zb8[:, hc, :], zT8["b"].ap()[:, hc, nb * 512 : (nb + 1) * 512]
                )
                nc.gpsimd.dma_start(
                    zbb[:, hc, :], zTb["b"].ap()[:, hc, nb * 512 : (nb + 1) * 512]
                )

        pT = {
            "a": wkvp.tile([128, DC, NL], BF16, name="pta"),
            "b": wkvp.tile([128, DC, NL], BF16, name="ptb"),
        }

        # per-direction fused AG buffers: [P_x | V_x] fp8
        agin = {x: dram.tile([SHARD], F8, name=f"agin_{x}") for x in ("b", "a")}
        agout = {
            x: dram.tile([R * SHARD], F8, name=f"agout_{x}", addr_space="Shared")
            for x in ("b", "a")
        }

        # ================= projection + P/V shards =================
        def mmtile(g=None):
            """single-bank accumulation tile; 5 rotating buffers so the PE
            never waits on a lagging Act/DVE consumer."""
            return psum.tile([128, 512], dt.float32, tag="mm", bufs=4, name="ps1")

        for src, other in (("a", "b"), ("b", "a")):
            for nb in range(NB):
                ns = slice(nb * 512, (nb + 1) * 512)
                z8, zb16 = zt[(src, nb)]
                hT = projp.tile([128, HC, 512], F32R, tag="h", bufs=2, name=f"h_{src}{nb}")
                # ELU(x)+1 = max(x+1, min(exp(x), 1)), x = ps + b1.
                # The combine (stt) is software-pipelined one block behind so
                # the DVE's PSUM reader (xp1) is never queued behind a combine
                # and the PSUM slot frees right after Act's Exp + DVE's add.
                stt_q = []
                for d1c in range(HC):
                    ps = mmtile()
                    for c in range(HC // 4):
                        nc.tensor.matmul(
                            ps,
                            w1t8[:, 2 * c : 2 * c + 2, d1c * 128 : (d1c + 1) * 128],
                            z8[:, 2 * c : 2 * c + 2, :],
                            start=(c == 0),
                            stop=False,
                            perf_mode=PM.DoubleRow,
                        )
                    for hc in range(HC // 2):
                        nc.tensor.matmul(
                            ps,
                            w1tb[:, hc, d1c * 128 : (d1c + 1) * 128],
                            zb16[:, hc, :],
                            start=False,
                            stop=(hc == HC // 2 - 1),
                        )
                    e = projp.tile([128, 512], dt.float32, tag="e", bufs=3, name="e")
                    xp1 = projp.tile([128, 512], dt.float32, tag="xp1", bufs=3, name="xp1")
                    # W1 shipped x16 in fp8; fold the 1/16 into the Act scale
                    nc.scalar.activation(
                        e[:], ps, AF.Exp, bias=b1[:, d1c : d1c + 1], scale=1.0 / 16
                    )
                    nc.scalar.activation(
                        xp1[:], ps, AF.Relu, bias=b1[:, d1c : d1c + 1], scale=1.0 / 16
                    )
                    stt_q.append((d1c, e, xp1))
                    if len(stt_q) > 1:
                        dd, ee, xx = stt_q.pop(0)
                        nc.vector.scalar_tensor_tensor(
                            hT[:, dd, :], ee[:], 1.0, xx[:], ALU.min, ALU.add
                        )
                dd, ee, xx = stt_q.pop(0)
                nc.vector.scalar_tensor_tensor(
                    hT[:, dd, :], ee[:], 1.0, xx[:], ALU.min, ALU.add
                )
                for d2c in range(DC):
                    ps = mmtile()
                    for d1c in range(HC):
                        nc.tensor.matmul(
                            ps,
                            w2t[:, d1c, d2c * 128 : (d2c + 1) * 128],
                            hT[:, d1c, :],
                            start=(d1c == 0),
                            stop=(d1c == HC - 1),
                        )
                    nc.scalar.activation(
                        pT[src][:, d2c, ns], ps, AF.Identity, bias=b2[:, d2c : d2c + 1]
                    )

            # stage this src's P (fp8, /4) + V into the *other* direction's
            # AG shard, then fire that direction's AllGather.
            pv = agin[other][0:KVF].rearrange("(d n) -> d n", n=NL)
            vv = agin[other][KVF : 2 * KVF].rearrange("(n d) -> n d", d=D)
            for dc in range(DC):
                for nb in range(NB):
                    ns = slice(nb * 512, (nb + 1) * 512)
                    s = projp.tile([128, 512], F8, tag="stg", bufs=16, name="stg_p")
                    nc.vector.tensor_scalar(
                        s[:], pT[src][:, dc, ns], 1.0 / QS, None, ALU.mult
                    )
                    nc.sync.dma_start(pv[dc * 128 : (dc + 1) * 128, ns], s[:])
            for nt in range(KC):
                ps = mmtile()
                for d2c in range(DC):
                    nc.tensor.matmul(
                        ps,
                        pT[src][:, d2c, nt * 128 : (nt + 1) * 128],
                        wvt[:, d2c, :],
                        start=(d2c == 0),
                        stop=(d2c == DC - 1),
                    )
                s = projp.tile([128, 512], F8, tag="stg", bufs=16, name="stg_v")
                nc.scalar.activation(s[:], ps, AF.Copy)
                nc.sync.dma_start(vv[nt * 128 : (nt + 1) * 128, :], s[:])
            nc.gpsimd.collective_compute(
                "AllGather",
                ALU.bypass,
                ins=[agin[other].opt()],
                outs=[agout[other].opt()],
                replica_groups=[list(range(R))],
            )

        # Pre-load direction-b shards r=0,1 into FRESH SBUF (attnp pool
        # created while projp is still live, so these tiles do not overlap
        # released projection space — otherwise their DMAs inherit an
        # anti-dependency on the last Q'' matmul and attention starts late).
        attnp = tc.alloc_tile_pool(name="attnp", bufs=1)
        pre = {}

        def load_shard(pool, x, r, eng=None):
            eng = eng or nc.sync
            base = r * SHARD
            ktile = pool.tile([128, DC, NL], F8, tag="kt", bufs=2, name=f"kt{r}")
            eng.dma_start(
                ktile[:],
                agout[x][base : base + KVF].rearrange(
                    "(dc p n) -> p dc n", p=128, n=NL
                ),
            )
            vtile = pool.tile([128, KC, D + 2], F8, tag="vt", bufs=3, name=f"vt{r}")
            eng.dma_start(
                vtile[:, :, 0:D],
                agout[x][base + KVF : base + 2 * KVF].rearrange(
                    "(kc p d) -> p kc d", p=128, d=D
                ),
            )
            eng.dma_start(
                vtile[:, :, D : D + 2],
                vpadd.ap().rearrange("p (kc c) -> p kc c", c=2),
            )
            return ktile, vtile

        for r in (0, 1):
            pre[("b", r)] = load_shard(attnp, "b", r, eng=nc.gpsimd)

        # ================= queries Q'' =================
        # Q''(b) is needed right away; Q''(a) is deferred until just before
        # direction-a attention so direction-b attention starts ~8us sooner.
        qT = {}

        def q_proj(x):
            qT[x] = qtp.tile([128, DC, NL], F8, name=f"qt_{x}")
            for dc in range(DC):
                for nb in range(NB):
                    ps = mmtile()
                    for d2c in range(DC):
                        nc.tensor.matmul(
                            ps,
                            wqkt[:, d2c, dc * 128 : (dc + 1) * 128],
                            pT[x][:, d2c, nb * 512 : (nb + 1) * 512],
                            start=(d2c == 0),
                            stop=(d2c == DC - 1),
                        )
                    nc.scalar.activation(
                        qT[x][:, dc, nb * 512 : (nb + 1) * 512],
                        ps,
                        AF.Identity,
                        bias=bqk[:, dc : dc + 1],
                    )

        q_proj("b")
        q_proj("a")

        # ================= attention (fp8 DoubleRow) =================
        # Software-pipelined: block i+1's score matmuls are issued before
        # block i's attn@V so the Exp activations never wait on PE.
        accs = {}
        pending = []  # deferred attn@V closures, one per (x, r, qb) block

        def do_scores(x, r, qb, ktile):
            qs = slice(qb * 512, (qb + 1) * 512)
            exps = []
            for kp in range(KC // 2):
                ex = attnp.tile([128, 2, 512], F8, tag="exp", bufs=16, name="ex")
                for j in range(2):
                    kt_i = 2 * kp + j
                    ps = mmtile()
                    for c in range(DC // 2):
                        nc.tensor.matmul(
                            ps,
                            ktile[:, 2 * c : 2 * c + 2,
                                  kt_i * 128 : (kt_i + 1) * 128],
                            qT[x][:, 2 * c : 2 * c + 2, qs],
                            start=(c == 0),
                            stop=(c == DC // 2 - 1),
                            perf_mode=PM.DoubleRow,
                        )
                    nc.scalar.activation(ex[:, j, :], ps, AF.Exp, bias=negc[:])
                exps.append(ex)
            return exps

        def do_attnv(x, col, r, qb, exps, vtile):
            for qt_i in range(4):
                qsl = slice(qt_i * 128, (qt_i + 1) * 128)
                p1 = psum.tile([128, 256], dt.float32, tag="po1", bufs=2, name="po1")
                p2 = psum.tile([128, 258], dt.float32, tag="po2", bufs=2, name="po2")
                for kp in range(KC // 2):
                    nc.tensor.matmul(
                        p1[:],
                        exps[kp][:, :, qsl],
                        vtile[:, 2 * kp : 2 * kp + 2, 0:256],
                        start=(kp == 0),
                        stop=(kp == KC // 2 - 1),
                        perf_mode=PM.DoubleRow,
                    )
                    nc.tensor.matmul(
                        p2[:],
                        exps[kp][:, :, qsl],
                        vtile[:, 2 * kp : 2 * kp + 2, 256 : D + 2],
                        start=(kp == 0),
                        stop=(kp == KC // 2 - 1),
                        perf_mode=PM.DoubleRow,
                    )
                if r == 0:
                    acc = attnp.tile(
                        [128, D + 2], dt.float32, tag="acc", bufs=8,
                        name=f"acc{qb}{qt_i}",
                    )
                    accs[(x, qb, qt_i)] = acc
                    nc.vector.tensor_copy(acc[:, 0:256], p1[:])
                    nc.vector.tensor_copy(acc[:, 256 : D + 2], p2[:])
                else:
                    acc = accs[(x, qb, qt_i)]
                    nc.vector.tensor_tensor(
                        acc[:, 0:256], acc[:, 0:256], p1[:], ALU.add
                    )
                    nc.vector.tensor_tensor(
                        acc[:, 256 : D + 2], acc[:, 256 : D + 2], p2[:], ALU.add
                    )
                if r == R - 1:
                    # finalize: out = acc[:, :512] / acc[:, 512] + bv.
                    # mult on Act (scale=rr), bv-add on DVE: splits the tail
                    # chain across two engines; out DMAs alternate rings.
                    rr = attnp.tile([128, 1], dt.float32, tag="rr", bufs=4, name="rr")
                    nc.vector.reciprocal(rr[:], acc[:, D : D + 1])
                    ot = attnp.tile([128, D], dt.float32, tag="ot", bufs=3, name="ot")
                    nc.vector.scalar_tensor_tensor(
                        ot[:], acc[:, 0:D], rr[:], bv[:], ALU.mult, ALU.add
                    )
                    r0 = qb * 512 + qt_i * 128
                    eng = nc.gpsimd if qt_i % 2 == 0 else nc.sync
                    eng.dma_start(
                        out_d.ap()[r0 : r0 + 128, col * D : (col + 1) * D], ot[:]
                    )

        for x, col in (("b", 1), ("a", 0)):
            for r in range(R):
                if (x, r) in pre:
                    ktile, vtile = pre[(x, r)]
                else:
                    ktile, vtile = load_shard(attnp, x, r)
                for qb in range(NB):
                    exps = do_scores(x, r, qb, ktile)
                    pending.append((x, col, r, qb, exps, vtile))
                    if len(pending) > 1:
                        do_attnv(*pending.pop(0))
        while pending:
            do_attnv(*pending.pop(0))
        attnp.release()
        projp.release()
        wkvp.release()
        qtp.release()
        const.release()
        dram.release()
        psum.release()

    nc.compile()
    return nc


_NC = None


def _get_nc():
    global _NC
    if _NC is None:
        _NC = build()
    return _NC


def _chunk_w(w):
    """[X, Y] -> [128, X//128, Y] partition-chunked, contiguous."""
    x, y = w.shape
    return np.ascontiguousarray(w.reshape(x // 128, 128, y).transpose(1, 0, 2))


def _chunk_b(b):
    return np.ascontiguousarray(np.asarray(b, np.float32).reshape(-1, 128).T)


def prep_in_maps(za, zb, W1, b1, W2, b2, Wq, bq, Wk, bk, Wv, bv):
    za = np.asarray(za, np.float32)
    zb = np.asarray(zb, np.float32)
    W1 = np.asarray(W1, np.float32)
    W2 = np.asarray(W2, np.float32)
    Wq = np.asarray(Wq, np.float32)
    Wk = np.asarray(Wk, np.float32)
    Wv = np.asarray(Wv, np.float32)
    b1 = np.asarray(b1, np.float32)
    b2 = np.asarray(b2, np.float32)
    bq = np.asarray(bq, np.float32)
    bk = np.asarray(bk, np.float32)
    bv = np.asarray(bv, np.float32)

    f8 = dt.np(F8)
    bf = dt.np(BF16)
    # Wk folded into the query side; 1/SCALE split as 1/QS per fp8 operand.
    Wqk = (Wq @ Wk.T) * (QS / SCALE)
    bqk = (bq @ Wk.T) * (QS / SCALE)
    shared = {
        "W1t8": np.ascontiguousarray(_chunk_w(W1 * 16.0)[:, : HC // 2]).astype(f8),
        "W1tb": np.ascontiguousarray(_chunk_w(W1 * 16.0)[:, HC // 2 :]).astype(bf),
        "W2t": _chunk_w(W2),
        "Wqkt": _chunk_w(Wqk).astype(bf),
        "Wvt": _chunk_w(Wv).astype(bf),
        "b1t": _chunk_b(b1),
        "b2t": _chunk_b(b2 - W2.sum(axis=0)),
        "bqkt": _chunk_b(bqk),
        "bvt": np.ascontiguousarray(np.broadcast_to(bv, (128, D)).astype(np.float32)),
        "vpad": np.ascontiguousarray(
            np.broadcast_to(
                np.tile(np.array([1.0, 0.0], np.float32), KC), (128, 2 * KC)
            )
        ).astype(f8),
    }
    zaT = np.ascontiguousarray(za.T)  # [H, N]
    zbT = np.ascontiguousarray(zb.T)
    in_maps = []
    for c in range(R):
        cs = slice(c * NL, (c + 1) * NL)
        in_maps.append(
            {
                "zaT8": np.ascontiguousarray(_chunk_w(zaT[:, cs])[:, : HC // 2]).astype(f8),
                "zaTb": np.ascontiguousarray(_chunk_w(zaT[:, cs])[:, HC // 2 :]).astype(bf),
                "zbT8": np.ascontiguousarray(_chunk_w(zbT[:, cs])[:, : HC // 2]).astype(f8),
                "zbTb": np.ascontiguousarray(_chunk_w(zbT[:, cs])[:, HC // 2 :]).astype(bf),
                **shared,
            }
        )
    return in_maps


def kernel(**inputs) -> np.ndarray:
    nc = _get_nc()
    in_maps = prep_in_maps(**inputs)
    res = run_bass_kernel_spmd(nc, in_maps, core_ids=list(range(R)))
    return np.concatenate([res.results[c]["out"] for c in range(R)], axis=0)
